# revision 35
# baseline (speedup 1.0000x reference)
"""Trainium2 Bass kernel for nn_DialogActLabeller (segment_reduce).

Computes, for input enc_output [32, 4096, 1024], W [1024, 256], b [256],
cls_pos [32, 64], last_sep [32]:

    x = enc_output @ W + b                      # [B, S, 256]
    seg[b, n] = sum_{s in [start_n, end_n)} x[b, s, :]
    out = log_softmax(seg, axis=-1)             # [B, 64, 256]

Key algebraic restructure: the projection is linear, so segment-reduce
FIRST on enc_output (via a matmul with a 0/1 segment-indicator matrix A),
then project the tiny per-batch result with W, and add len_n * b (as a
rank-1 matmul into the same PSUM accumulator).  This reads enc_output
exactly once from HBM and does ~1/32 of the naive FLOPs.

The kernel is HBM-bandwidth bound on the enc_output read, so enc is
shipped as fp8 (e4m3) — 4x less DMA than fp32.  Plain elementwise fp8
rounding would lose too much precision in long segment sums, so the host
quantizes with error feedback (sigma-delta) along the sequence axis:
    q[s] = fp8(enc[s] + c),  c += enc[s] - q[s]
which makes every segment sum of q match the segment sum of enc to
within one quantization step at each boundary, independent of segment
length.  The segment-reduce matmul then runs in fp8 DoubleRow perf mode
(2 contraction rows per cycle).

Schedule: batches are processed in pairs — each batch's seg result is
transposed (bf16, 8 PE transposes) into the free-dim half of a shared
[128, 8, 128] PSUM tile, so the projection runs once per PAIR at full
128-partition PE utilization, as does the softmax front half (fused
exp+sum on the ACT engine).  Tails are emitted after the next batch's
seg matmuls so the tensor queue never bubbles.  A single Ln + subtract
+ one output DMA form the only non-overlapped tail.

Sharding: pure data parallel, 4 batch rows per core across 8 cores
(W, b replicated), no cross-core communication.
"""

import numpy as np
import ml_dtypes

import concourse.bacc as bacc
import concourse.bass as bass
import concourse.tile as tile
from concourse import mybir
from concourse import bass_utils
from contextlib import ExitStack

# Problem shapes (hardcoded per contract)
B, S, D_IN, D_OUT, N_SENT = 32, 4096, 1024, 256, 64
N_CORES = 8
BPC = B // N_CORES          # batches per core
NPAIR = BPC // 2            # batch pairs per core
SCHUNKS = S // 128          # 32 sequence subtiles of 128
DCH = D_IN // 128           # 8 d_in chunks of 128
# per-batch enc DMA chunk sizes in s-subtiles (batch 0 starts small so the
# first matmul fires as early as possible during the DMA ramp; 8-subtile
# chunks = 8 KiB per-partition lines, which the DMA engines need to sustain
# full HBM rate)
CHUNKS_B0 = [2, 2, 4, 8, 8, 8]
CHUNKS = [8] * 4
ENC_BUFS = {2: 2, 4: 1, 8: 10}

F32 = mybir.dt.float32
F32R = mybir.dt.float32r
BF16 = mybir.dt.bfloat16
U32 = mybir.dt.uint32
FP8 = mybir.dt.float8e4
E4M3 = ml_dtypes.float8_e4m3   # numpy view of TRN FP8_EXP4

# degree-3 fit of ln(x) on [1, 2] for the DVE bit-trick log (the ACT Ln
# table load costs 1.3us on the critical tail; ssum is always in [1, 257))
_xs = np.linspace(1.0, 2.0, 4097)
_A3, _A2, _A1, _A0 = np.polyfit(_xs, np.log(_xs), 3).tolist()
_LN2 = 0.6931471805599453


def _build_program():
    nc = bacc.Bacc("TRN2", debug=False)

    # enc host-pre-tiled to [BPC, 128, SCHUNKS*D_IN] fp8 with s = t*128 + p,
    # so ANY run of s-subtiles is one contiguous per-partition byte range.
    enc = nc.dram_tensor(
        "enc", [BPC, 128, SCHUNKS * D_IN], FP8, kind="ExternalInput"
    ).ap()
    # W host-pre-tiled to [128, DCH*D_OUT] bf16 with layout [p, j, o]
    wt = nc.dram_tensor("w", [128, DCH * D_OUT], BF16, kind="ExternalInput").ap()
    bias = nc.dram_tensor("bias", [D_OUT], F32R, kind="ExternalInput").ap()
    # 0/1 segment-indicator matrices in fp8, amat[b, p, k*N+n] = A[s=k*128+p, n]
    amat = nc.dram_tensor(
        "amat", [BPC, 128, SCHUNKS * N_SENT], FP8, kind="ExternalInput"
    ).ap()
    # lensT[pr, h*64+n] = segment length of (batch 2*pr+h, sentence n)
    lensT = nc.dram_tensor("lensT", [NPAIR, 128], F32R, kind="ExternalInput").ap()
    identb = nc.dram_tensor("identb", [N_SENT, N_SENT], BF16,
                            kind="ExternalInput").ap()
    # out in the paired layout [128(h*64+n), pair, 256]; host unshuffles
    out = nc.dram_tensor(
        "out", [128, NPAIR, D_OUT], F32, kind="ExternalOutput"
    ).ap()

    with tile.TileContext(nc) as tc, ExitStack() as ctx:
        singles = ctx.enter_context(tc.tile_pool(name="singles", bufs=1))
        encp = ctx.enter_context(tc.tile_pool(name="encp", bufs=14))
        segp = ctx.enter_context(tc.tile_pool(name="segp", bufs=2))
        smalls = ctx.enter_context(tc.tile_pool(name="smalls", bufs=1))
        ps_seg = ctx.enter_context(tc.tile_pool(name="ps_seg", bufs=2, space="PSUM"))
        ps_tr = ctx.enter_context(tc.tile_pool(name="ps_tr", bufs=2, space="PSUM"))
        ps_pr = ctx.enter_context(tc.tile_pool(name="ps_pr", bufs=2, space="PSUM"))

        # ---- PE warm-up: the tensor engine boots at its lowest p-state and
        # only speeds up with sustained activity.  Run a few dummy DoubleRow
        # matmuls on zeroed tiles during the otherwise-dead DMA ramp so the
        # real batch-0 matmuls start warm. ----
        zt = singles.tile([128, 2, 576], FP8)
        nc.gpsimd.memset(zt, 0)
        warm_ps = ps_seg.tile([N_SENT, 512], F32, tag="ps0", name="warm")
        for _ in range(7):
            nc.tensor.matmul(
                warm_ps,
                lhsT=zt[:, :, 0:N_SENT],
                rhs=zt[:, :, N_SENT: N_SENT + 512],
                start=True,
                stop=True,
                perf_mode=mybir.MatmulPerfMode.DoubleRow,
            )

        # per-batch fp8 A tiles. a8_0 is the FIRST transfer on the scalar
        # ring so the first seg matmul never waits on the const stream.
        a8_t = [
            singles.tile([128, SCHUNKS, N_SENT], FP8, tag=f"a8_{bi}",
                         name=f"a8_{bi}")
            for bi in range(BPC)
        ]
        nc.scalar.dma_start(
            out=a8_t[0], in_=amat[0].rearrange("p (k n) -> p k n", n=N_SENT)
        )

        # ---- constants (ACT HWDGE ring; only needed by the first pair tail,
        # ~20us in, so they never gate the seg matmul stream) ----
        w_sb = singles.tile([128, DCH, D_OUT], BF16)
        nc.scalar.dma_start(out=w_sb, in_=wt.rearrange("p (j o) -> p j o", o=D_OUT))
        ident_sb = singles.tile([N_SENT, N_SENT], BF16)
        nc.scalar.dma_start(out=ident_sb, in_=identb)
        # lens rows on partition 0, as lhsT of the rank-1 len*b matmul
        lensT_sb = singles.tile([1, NPAIR, 128], F32R)
        nc.scalar.dma_start(out=lensT_sb, in_=lensT.rearrange("r n -> (r n)"))
        b1_sb = singles.tile([1, D_OUT], F32R)
        nc.scalar.dma_start(out=b1_sb, in_=bias)

        # softmax staging in the paired layout (2 batches per partition set)
        ssum_all = smalls.tile([128, NPAIR], F32, tag="ssum")
        negmax_all = smalls.tile([128, NPAIR], F32, tag="negmax")
        ex_scr = smalls.tile([128, D_OUT], F32, tag="ex")
        ot_all = singles.tile([128, NPAIR, D_OUT], F32)

        def emit_seg(bi, part, ps0, ps1):
            """Enc DMA stream + fp8 DoubleRow seg-reduce matmuls for batch bi.
            part 0 = first two chunks, part 1 = the rest, so other tensor-queue
            work can be interleaved mid-batch (its dependencies are then
            guaranteed met and the queue never stalls)."""
            chunks = CHUNKS_B0 if bi == 0 else CHUNKS
            split = 2
            sel = chunks[:split] if part == 0 else chunks[split:]
            if part == 0:
                if bi > 0:
                    nc.scalar.dma_start(
                        out=a8_t[bi],
                        in_=amat[bi].rearrange("p (k n) -> p k n", n=N_SENT),
                    )
                t0 = 0
                j = 0
            else:
                t0 = sum(chunks[:split])
                j = t0 // 2
            enc_b = enc[bi].rearrange("p (t d) -> p t d", d=D_IN)
            for csz in sel:
                et = encp.tile([128, csz, D_IN], FP8, tag=f"enc{csz}",
                               name=f"enc_{bi}_{t0}", bufs=ENC_BUFS[csz])
                # alternate full-size chunks across the two HWDGE rings so
                # queue-side trigger/semaphore latency doesn't gate the stream
                ring = nc.scalar if (csz == 8 and (t0 // 8) % 2 == 1) else nc.sync
                ring.dma_start(out=et, in_=enc_b[:, t0: t0 + csz, :])
                for tp in range(csz // 2):
                    lhsT = a8_t[bi][:, t0 + 2 * tp: t0 + 2 * tp + 2, :]
                    for dh in range(2):
                        rhs = et[:, 2 * tp: 2 * tp + 2, dh * 512: (dh + 1) * 512]
                        nc.tensor.matmul(
                            ps0 if dh == 0 else ps1,
                            lhsT=lhsT,
                            rhs=rhs,
                            start=(j == 0),
                            stop=(j == SCHUNKS // 2 - 1),
                            perf_mode=mybir.MatmulPerfMode.DoubleRow,
                        )
                    j += 1
                t0 += csz

        def alloc_ps(bi):
            ps0 = ps_seg.tile([N_SENT, 512], F32, tag="ps0", name=f"ps0_{bi}")
            ps1 = ps_seg.tile([N_SENT, 512], F32, tag="ps1", name=f"ps1_{bi}")
            return ps0, ps1

        def emit_drain(bi, ps0, ps1):
            """psum -> bf16 seg in SBUF, as soon as batch bi's matmuls end.
            One half on DVE, one on ACT so they run in parallel."""
            seg_sb = segp.tile([N_SENT, D_IN], BF16, tag="seg", name=f"seg_{bi}")
            nc.vector.tensor_copy(out=seg_sb[:, 0:512], in_=ps0)
            nc.scalar.activation(
                out=seg_sb[:, 512:1024], in_=ps1,
                func=mybir.ActivationFunctionType.Copy,
            )
            return seg_sb

        def emit_transposes(bi, seg_sb, pt):
            """8 PE transposes of batch bi's seg into its free-dim half of
            the pair's shared [128, 8, 128] psum tile."""
            h = (bi % 2) * N_SENT
            for j in range(DCH):
                nc.tensor.transpose(
                    out=pt[:, j, h: h + N_SENT],
                    in_=seg_sb[:, j * 128: (j + 1) * 128],
                    identity=ident_sb,
                )

        def emit_pair_cast(pr, pt):
            seg_t = segp.tile([128, DCH, 128], BF16, tag="segT", name=f"sgt_{pr}")
            nc.vector.tensor_copy(out=seg_t[:, 0:4, :], in_=pt[:, 0:4, :])
            nc.vector.tensor_copy(out=seg_t[:, 4:8, :], in_=pt[:, 4:8, :])
            return seg_t

        def emit_pair_proj(pr, seg_t):
            """len*b rank-1 matmul + 8 paired projection matmuls + the whole
            softmax: fused exp+sum on ACT, then a DVE bit-trick log (no ACT
            table load), subtract, and this pair's 128 KiB output DMA."""
            pp = ps_pr.tile([128, D_OUT], F32, tag="pp", name=f"pp_{pr}")
            nc.tensor.matmul(
                pp,
                lhsT=lensT_sb[:, pr, :],
                rhs=b1_sb,
                start=True,
                stop=False,
            )
            for j in range(DCH):
                nc.tensor.matmul(
                    pp,
                    lhsT=seg_t[:, j, :],
                    rhs=w_sb[:, j, :],
                    start=False,
                    stop=(j == DCH - 1),
                )
            negmax = negmax_all[:, pr: pr + 1]
            nc.vector.tensor_reduce(
                out=negmax, in_=pp,
                axis=mybir.AxisListType.X,
                op=mybir.AluOpType.max, negate=True,
            )
            # one fused ACT op: ex = exp(sv + negmax), ssum = sum(ex)
            nc.scalar.activation(
                out=ex_scr, in_=pp,
                func=mybir.ActivationFunctionType.Exp,
                bias=negmax,
                accum_out=ssum_all[:, pr: pr + 1],
            )
            # lse = ln(ssum), ssum in [1, 257): exponent bits + deg-3 poly
            # via fused (x + c) * m Horner steps on the DVE (no ACT Ln table)
            u = ssum_all[:, pr: pr + 1].bitcast(U32)
            e_u = smalls.tile([128, 1], U32, tag=f"eu{pr}", name=f"eu{pr}")
            nc.vector.tensor_scalar(
                out=e_u, in0=u, scalar1=23, scalar2=None,
                op0=mybir.AluOpType.logical_shift_right,
            )
            e_f = smalls.tile([128, 1], F32, tag=f"ef{pr}", name=f"ef{pr}")
            nc.vector.tensor_copy(out=e_f, in_=e_u)
            mb = smalls.tile([128, 1], U32, tag=f"mb{pr}", name=f"mb{pr}")
            nc.vector.tensor_scalar(
                out=mb, in0=u, scalar1=0x007FFFFF, scalar2=0x3F800000,
                op0=mybir.AluOpType.bitwise_and, op1=mybir.AluOpType.bitwise_or,
            )
            m = mb.bitcast(F32)
            p = smalls.tile([128, 1], F32, tag=f"pl{pr}", name=f"pl{pr}")
            # p = (m + A2/A3) * m ; p = (p + A1/A3) * m
            nc.vector.scalar_tensor_tensor(
                out=p, in0=m, scalar=_A2 / _A3, in1=m,
                op0=mybir.AluOpType.add, op1=mybir.AluOpType.mult,
            )
            nc.vector.scalar_tensor_tensor(
                out=p, in0=p, scalar=_A1 / _A3, in1=m,
                op0=mybir.AluOpType.add, op1=mybir.AluOpType.mult,
            )
            # p = A3*p + (A0 - 127*ln2)
            nc.vector.tensor_scalar(
                out=p, in0=p, scalar1=_A3, scalar2=_A0 - 127.0 * _LN2,
                op0=mybir.AluOpType.mult, op1=mybir.AluOpType.add,
            )
            # nl = negmax - (ln2*e + p);  out = sv + nl
            lse = smalls.tile([128, 1], F32, tag=f"ls{pr}", name=f"ls{pr}")
            nc.vector.scalar_tensor_tensor(
                out=lse, in0=e_f, scalar=_LN2, in1=p,
                op0=mybir.AluOpType.mult, op1=mybir.AluOpType.add,
            )
            nl = smalls.tile([128, 1], F32, tag=f"nl{pr}", name=f"nl{pr}")
            nc.vector.tensor_tensor(
                out=nl, in0=negmax, in1=lse, op=mybir.AluOpType.subtract,
            )
            nc.vector.tensor_scalar(
                out=ot_all[:, pr, :], in0=pp,
                scalar1=nl, scalar2=None, op0=mybir.AluOpType.add,
            )
            # out DMA on the ACT ring: the sync ring is pure enc stream, so
            # this never stalls later enc chunk triggers behind the DVE chain
            nc.scalar.dma_start(out=out[:, pr, :], in_=ot_all[:, pr, :])

        # ---- software-pipelined main loop over batch pairs.  T(b) and the
        # pair finishes are tucked into the MIDDLE of the next batch's seg
        # stream so their DVE-side dependencies are already met when the
        # tensor queue reaches them. ----
        pt0 = ps_tr.tile([128, DCH, 128], BF16, tag="pt", name="pt_0")
        pt1 = ps_tr.tile([128, DCH, 128], BF16, tag="pt", name="pt_1")
        ps_b0 = alloc_ps(0)
        emit_seg(0, 0, *ps_b0)
        # keep the PE warm (p-state) while the DMA ramp delivers chunk 2
        for _ in range(5):
            nc.tensor.matmul(
                warm_ps,
                lhsT=zt[:, :, 0:N_SENT],
                rhs=zt[:, :, N_SENT: N_SENT + 512],
                start=True,
                stop=True,
                perf_mode=mybir.MatmulPerfMode.DoubleRow,
            )
        emit_seg(0, 1, *ps_b0)
        sb0 = emit_drain(0, *ps_b0)
        ps_b1 = alloc_ps(1)
        emit_seg(1, 0, *ps_b1)
        emit_transposes(0, sb0, pt0)
        emit_seg(1, 1, *ps_b1)
        sb1 = emit_drain(1, *ps_b1)
        ps_b2 = alloc_ps(2)
        emit_seg(2, 0, *ps_b2)
        emit_transposes(1, sb1, pt0)
        emit_seg(2, 1, *ps_b2)
        sb2 = emit_drain(2, *ps_b2)
        st0 = emit_pair_cast(0, pt0)
        ps_b3 = alloc_ps(3)
        emit_seg(3, 0, *ps_b3)
        emit_pair_proj(0, st0)
        emit_transposes(2, sb2, pt1)
        emit_seg(3, 1, *ps_b3)
        sb3 = emit_drain(3, *ps_b3)
        emit_transposes(3, sb3, pt1)
        st1 = emit_pair_cast(1, pt1)
        emit_pair_proj(1, st1)

    nc.compile()
    return nc


_PROGRAM = None


def _get_program():
    global _PROGRAM
    if _PROGRAM is None:
        _PROGRAM = _build_program()
    return _PROGRAM


def _ef_quantize(enc):
    """Sigma-delta quantize enc [B, S, D] fp32 -> fp8 e4m3 along axis 1.

    Error feedback keeps every prefix sum of q within one fp8 quantization
    step of the true prefix sum, so segment sums stay accurate regardless
    of segment length.
    """
    q8 = np.empty(enc.shape, E4M3)
    c = np.zeros((enc.shape[0], enc.shape[2]), np.float32)
    for s in range(enc.shape[1]):
        v = enc[:, s, :] + c
        qs = v.astype(E4M3)
        q8[:, s, :] = qs
        c = v - qs.astype(np.float32)
    return q8


def _host_prep(enc_output, W, b, cls_pos, last_sep):
    enc = np.asarray(enc_output, dtype=np.float32)
    q8 = _ef_quantize(enc)
    # pre-tile so any s-subtile run is contiguous per partition:
    # [B, S, D] -> [B, 128(p), SCHUNKS(t) * D]  with s = t*128 + p
    q8 = np.ascontiguousarray(
        q8.reshape(B, SCHUNKS, 128, D_IN)
        .transpose(0, 2, 1, 3)
        .reshape(B, 128, SCHUNKS * D_IN)
    )
    wf = np.asarray(W, dtype=np.float32)
    # [D_IN, D_OUT] -> [128(p), DCH(j) * D_OUT] bf16 with d = j*128+p
    wf = np.ascontiguousarray(
        wf.reshape(DCH, 128, D_OUT).transpose(1, 0, 2).reshape(128, DCH * D_OUT)
    ).astype(ml_dtypes.bfloat16)
    bf = np.ascontiguousarray(np.asarray(b, dtype=np.float32))
    starts = np.asarray(cls_pos).astype(np.int64)                    # [B, N]
    lsep = np.asarray(last_sep).astype(np.int64)                     # [B]
    ends = np.concatenate([starts[:, 1:], (lsep + 1)[:, None]], axis=1)
    # torch semantics for the last segment: if end <= start, sum to seq end
    ends[:, -1] = np.where(ends[:, -1] > starts[:, -1], ends[:, -1], S)
    lens = (ends - starts).astype(np.float32)                        # [B, N]
    # paired layout per core: lensT[c, pr, h*64+n] = lens[c*BPC + 2*pr + h, n]
    lensT = np.ascontiguousarray(
        lens.reshape(N_CORES, NPAIR, 2 * N_SENT)
    )

    s = np.arange(S, dtype=np.int64)
    afull = (s[None, :, None] >= starts[:, None, :]) & (
        s[None, :, None] < ends[:, None, :]
    )                                                                # [B, S, N]
    amat = (
        afull.reshape(B, SCHUNKS, 128, N_SENT)
        .transpose(0, 2, 1, 3)
        .reshape(B, 128, SCHUNKS * N_SENT)
        .astype(np.uint8)
        .astype(E4M3)
    )
    return q8, wf, bf, amat, lensT


def kernel(enc_output, W, b, max_num_sent, cls_pos, last_sep, _trace=False):
    q8, wf, bf, amat, lensT = _host_prep(enc_output, W, b, cls_pos, last_sep)
    identb = np.eye(N_SENT, dtype=np.float32).astype(ml_dtypes.bfloat16)

    nc = _get_program()
    in_maps = []
    for c in range(N_CORES):
        bsl = slice(c * BPC, (c + 1) * BPC)
        in_maps.append(
            {
                "enc": q8[bsl],
                "w": wf,
                "bias": bf,
                "amat": amat[bsl],
                "lensT": lensT[c],
                "identb": identb,
            }
        )
    res = bass_utils.run_bass_kernel_spmd(
        nc, in_maps, core_ids=list(range(N_CORES)), trace=_trace
    )
    # device out is [128(h*64+n), pair, 256] per core; unshuffle to [B, N, O]
    out = np.stack([res.results[c]["out"] for c in range(N_CORES)], axis=0)
    out = (
        out.reshape(N_CORES, 2, N_SENT, NPAIR, D_OUT)
        .transpose(0, 3, 1, 2, 4)
        .reshape(B, N_SENT, D_OUT)
    )
    if _trace:
        kernel._last_result = res
    return np.ascontiguousarray(out).astype(np.float32)
